# revision 32
# baseline (speedup 1.0000x reference)
"""Trainium2 Bass kernel for nn_ConditionalLayer (moe_routing).

out[i] = x[i] @ W[cond[i]].T + b.sum(0)       x:[8192,1024] W:[16,1024,1024]

Strategy (expert-parallel, host-routed):
  - Host groups rows by cond value (the "shard the condition axis" strategy):
    each of the 8 cores owns 2 of the 16 experts and receives only the rows
    routed to them, padded per expert slot to a multiple of 128.
  - Host pre-transposes x and W so both matmul operands have the contraction
    dim (d) on SBUF partitions -> no on-device transposes.
  - Device: per row-tile [128 rows] x f-block [512 cols]: 8 accumulating
    matmuls over d-chunks; bias b.sum(0) fused into the PSUM->SBUF eviction.
  - Host scatters routed rows back to their original positions.
"""

import os
import sys

import numpy as np

_TRN_REPO = "/opt/trn_rl_repo"
if os.path.isdir(_TRN_REPO) and _TRN_REPO not in sys.path:
    sys.path.insert(0, _TRN_REPO)

B, D, C = 8192, 1024, 16
NCORES = 8
SLOTS = C // NCORES  # experts per core
P = 128
FBLK = 512  # psum bank / fp32 moving-operand limit
DK = D // P  # contraction chunks
NF = D // FBLK  # f blocks

# 'float32' (exact, PE at 1/4 rate) or 'float32r' (tf32-like, full rate)
MM_DTYPE = "float32r"
TRACE = False
LAST_RESULT = None
LAST_NC = None

_nc_cache = {}


def _make_tile_context_cls():
    import concourse.mybir as mybir
    from concourse import tile
    from concourse.vector_clock import ScopedClock

    class TileContextFix(tile.TileContext):
        """This walrus build rejects >1 sync-wait per instruction.  Tile's
        scheduler freely assigns several.  Split the extras onto preceding
        NOPs on the same engine (same-engine program order makes this
        equivalent), and likewise chain the tail drain's waits."""

        _ws_counter = 0

        def _split_multi_waits(self):
            nc = self.nc
            for bb in nc.m.functions[0].blocks:
                insts = list(bb.instructions)
                if not any(
                    i.sync_info
                    and i.sync_info.on_wait
                    and len(i.sync_info.on_wait) > 1
                    for i in insts
                ):
                    continue
                new_seq = []
                for inst in insts:
                    si = inst.sync_info
                    waits = (
                        list(si.on_wait) if (si is not None and si.on_wait) else []
                    )
                    if len(waits) > 1:
                        for w in waits[:-1]:
                            TileContextFix._ws_counter += 1
                            nop = mybir.InstNoOp(
                                name=f"I-waitsplit-{TileContextFix._ws_counter}",
                                engine=inst.engine,
                            )
                            nop.sync_info = mybir.SyncInfo(
                                on_wait=[w], on_update=[]
                            )
                            new_seq.append(nop)
                        inst.sync_info = mybir.SyncInfo(
                            on_wait=[waits[-1]],
                            on_update=list(si.on_update) if si.on_update else [],
                        )
                    new_seq.append(inst)
                bb.instructions[:] = new_seq

        def _drain_and_barrier(self, tick_clock, wait_clock):
            self._split_multi_waits()
            drain_inst = self.nc.sync.drain()
            wait_clock.add_sem_waits(
                drain_inst.ins, ScopedClock({None: tick_clock.global_clock})
            )
            si = drain_inst.ins.sync_info
            waits = list(si.on_wait) if si is not None else []
            if len(waits) > 1:
                drain_inst.ins.sync_info = mybir.SyncInfo(
                    on_wait=waits[:1],
                    on_update=list(si.on_update) if si.on_update else [],
                )
                for w in waits[1:]:
                    extra = self.nc.sync.drain()
                    extra.ins.sync_info = mybir.SyncInfo(on_wait=[w], on_update=[])
            self.nc.all_engine_barrier()
            assert self.sems is not None
            popped = self.nc._tile_sem_poison_stack.pop()
            assert popped is self._sem_poison
            self.nc.clear_and_free_semaphores(list(self.sems.allocated().values()))
            self.nc.all_engine_barrier()

    return TileContextFix


def _tiles_of(M0, M1):
    """Row-tiles [(slot, col0, rows)]: full 128-row tiles + one ragged
    tile per slot."""
    tiles = []
    for s, (base, M) in enumerate(((0, M0), (M0, M1))):
        c = 0
        while c < M:
            r = min(P, M - c)
            tiles.append((s, base + c, r))
            c += r
    return tiles


def _build(M0, M1, mm_dtype):
    key = (M0, M1, mm_dtype)
    if key in _nc_cache:
        return _nc_cache[key]

    import concourse.bass as bass
    import concourse.mybir as mybir

    TileContextFix = _make_tile_context_cls()

    NTOT = M0 + M1
    # x columns padded so every 128-wide tile load stays in bounds
    NXPAD = M0 + P * (-(-M1 // P))
    nc = bass.Bass()
    mmdt = getattr(mybir.dt, mm_dtype)
    # x^T: [d, n] with routed rows as columns (slot0 block then slot1);
    # W pre-tiled on host as [slot][fb][dk] 128x512 contiguous blocks
    xT = nc.declare_dram_parameter("xT", [D, NXPAD], mmdt, isOutput=False)
    wt = nc.declare_dram_parameter(
        "wt", [SLOTS, NF, DK, P, FBLK], mmdt, isOutput=False
    )
    bias = nc.declare_dram_parameter("bias", [P, D], mybir.dt.float32, isOutput=False)
    out = nc.declare_dram_parameter("out", [NTOT, D], mybir.dt.float32, isOutput=True)

    tiles = _tiles_of(M0, M1)
    n_slot0 = sum(1 for s, _, _ in tiles if s == 0)
    WSPLIT = globals().get("_WSPLIT", 1)  # whole-W DMAs schedule best

    with TileContextFix(nc) as tc:
        with (
            tc.tile_pool(name="wpool", bufs=1) as wpool,
            tc.tile_pool(name="xpool", bufs=1) as xpool,
            tc.tile_pool(name="bpool", bufs=1) as bpool,
            tc.tile_pool(name="psum", bufs=6, space="PSUM") as pp,
            tc.tile_pool(name="opool", bufs=4) as op,
        ):
            # HWDGE descriptor generation is a serial ~625ns/DMA resource:
            # batch aggressively.  One DMA per x row-tile (all dk chunks,
            # always 128 cols -> full-width 512B descriptors), W in 1MB
            # half-blocks, one (ragged) store per row-tile.
            x_tiles = {}

            def load_x(t):
                _, c0, r = tiles[t]
                tl = xpool.tile([P, DK * P], mmdt, tag=f"x{t}")
                src = xT[:, c0 : c0 + P].rearrange("(dk p) m -> p dk m", p=P)
                nc.sync.dma_start(tl[:], src)
                x_tiles[t] = tl

            w_tiles = {}

            def load_w_half(s, fb, i):
                step = DK // WSPLIT
                tl = wpool.tile([P, step * FBLK], mmdt, tag=f"w{s}_{fb}_{i}")
                nc.sync.dma_start(
                    tl[:],
                    wt[s, fb, i * step : (i + 1) * step].rearrange(
                        "dk p f -> p dk f"
                    ),
                )
                w_tiles.setdefault((s, fb), [None] * WSPLIT)[i] = tl

            def load_w(s, fb):
                for i in range(WSPLIT):
                    load_w_half(s, fb, i)

            def w_slice(s, fb, dk):
                step = DK // WSPLIT
                tl = w_tiles[(s, fb)][dk // step]
                d = dk % step
                return tl[:, d * FBLK : (d + 1) * FBLK]

            bias_t = bpool.tile([P, D], mybir.dt.float32, tag="bias")

            # issue order = pipeline order: first psum group needs w(0,0)
            # and x(0); slot-1 weights land before the PE reaches the
            # slot-1 tiles, so the PE ramps early and stays fed.
            # bias first: the DVE evictions read bias_t, so it must be
            # resident before the first psum group retires or the psum
            # pool backs up and stalls the PE; later placements measure
            # strictly worse
            nc.sync.dma_start(bias_t[:], bias[:])
            load_w(0, 0)
            load_x(0)
            load_x(1)
            load_w(0, 1)
            for t in range(2, min(n_slot0 + 1, len(tiles))):
                load_x(t)
            load_w(1, 0)
            load_w(1, 1)
            for t in range(n_slot0 + 1, len(tiles)):
                load_x(t)

            for t, (s, c0, r) in enumerate(tiles):
                ot = op.tile([P, D], mybir.dt.float32, tag="o")
                for fb in range(NF):
                    ps = pp.tile([P, FBLK], mybir.dt.float32, tag="ps")
                    for dk in range(DK):
                        nc.tensor.matmul(
                            ps[:r, :],
                            x_tiles[t][:, dk * P : dk * P + r],
                            w_slice(s, fb, dk),
                            start=(dk == 0),
                            stop=(dk == DK - 1),
                        )
                    nc.vector.tensor_add(
                        ot[:r, fb * FBLK : (fb + 1) * FBLK],
                        ps[:r, :],
                        bias_t[:r, fb * FBLK : (fb + 1) * FBLK],
                    )
                # store issued from the otherwise-idle ACT engine so its
                # waits never head-of-line block the SP load stream
                nc.scalar.dma_start(out[c0 : c0 + r, :], ot[:r, :])

    _nc_cache[key] = nc
    return nc


def kernel(x, cond, W, b):
    from concourse.bass_utils import run_bass_kernel_spmd

    global LAST_RESULT, LAST_NC

    x = np.ascontiguousarray(np.asarray(x, dtype=np.float32))
    cond_i = np.asarray(cond).astype(np.int64)
    W = np.asarray(W, dtype=np.float32)
    b = np.asarray(b, dtype=np.float32)

    counts = np.bincount(cond_i, minlength=C)
    # Largest 8 experts -> slot 0, rest -> slot 1, so per-slot padding
    # (max count over that slot) is minimal.
    order = np.argsort(-counts, kind="stable")
    slot_experts = (order[:NCORES], order[NCORES:])
    M0 = max(1, int(counts[slot_experts[0]].max()))
    M1 = max(1, int(counts[slot_experts[1]].max()))
    NXPAD = M0 + P * (-(-M1 // P))

    nc = _build(M0, M1, MM_DTYPE)
    LAST_NC = nc

    bias_np = np.ascontiguousarray(
        np.broadcast_to(b.sum(axis=0).astype(np.float32), (P, D))
    )

    idx_by_e = [np.nonzero(cond_i == e)[0] for e in range(C)]
    in_maps = []
    placements = []
    for k in range(NCORES):
        xTk = np.zeros((D, NXPAD), np.float32)
        wTk = np.empty((SLOTS, D, D), np.float32)
        for s, col in enumerate((0, M0)):
            e = int(slot_experts[s][k])
            idx = idx_by_e[e]
            xTk[:, col : col + len(idx)] = x[idx].T
            wTk[s] = W[e].T
            placements.append((k, col, e))
        # [S, D, D] -> [S, NF, DK, 128, 512] contiguous blocks
        wtk = np.ascontiguousarray(
            wTk.reshape(SLOTS, DK, P, NF, FBLK).transpose(0, 3, 1, 2, 4)
        )
        in_maps.append({"xT": xTk, "wt": wtk, "bias": bias_np})

    res = run_bass_kernel_spmd(
        nc, in_maps, list(range(NCORES)), trace=TRACE
    )
    LAST_RESULT = res

    out_full = np.empty((B, D), np.float32)
    for k, col, e in placements:
        idx = idx_by_e[e]
        out_full[idx] = res.results[k]["out"][col : col + len(idx)]
    return out_full


if __name__ == "__main__":
    rng = np.random.default_rng(0)
    x = rng.standard_normal((B, D), dtype=np.float32)
    cond = rng.integers(0, C, size=B).astype(np.int64)
    W = (rng.standard_normal((C, D, D), dtype=np.float32) / np.sqrt(D)).astype(
        np.float32
    )
    b = (rng.standard_normal((C, D), dtype=np.float32) * 0.02).astype(np.float32)
    got = kernel(x, cond, W, b)
    want = np.empty((B, D), np.float32)
    for e in range(C):
        idx = np.nonzero(cond == e)[0]
        want[idx] = x[idx] @ W[e].T
    want += b.sum(0)
    denom = np.abs(want).max()
    print("max abs err:", np.abs(got - want).max(), "denom:", denom)
    print("rel err:", np.abs(got - want).max() / denom)


# revision 35
# speedup vs baseline: 1.0102x; 1.0102x over previous
"""Trainium2 Bass kernel for nn_ConditionalLayer (moe_routing).

out[i] = x[i] @ W[cond[i]].T + b.sum(0)       x:[8192,1024] W:[16,1024,1024]

Strategy (expert-parallel, host-routed):
  - Host groups rows by cond value (the "shard the condition axis" strategy):
    each of the 8 cores owns 2 of the 16 experts and receives only the rows
    routed to them, padded per expert slot to a multiple of 128.
  - Host pre-transposes x and W so both matmul operands have the contraction
    dim (d) on SBUF partitions -> no on-device transposes.
  - Device: per row-tile [128 rows] x f-block [512 cols]: 8 accumulating
    matmuls over d-chunks; bias b.sum(0) fused into the PSUM->SBUF eviction.
  - Host scatters routed rows back to their original positions.
"""

import os
import sys

import numpy as np

_TRN_REPO = "/opt/trn_rl_repo"
if os.path.isdir(_TRN_REPO) and _TRN_REPO not in sys.path:
    sys.path.insert(0, _TRN_REPO)

B, D, C = 8192, 1024, 16
NCORES = 8
SLOTS = C // NCORES  # experts per core
P = 128
FBLK = 512  # psum bank / fp32 moving-operand limit
DK = D // P  # contraction chunks
NF = D // FBLK  # f blocks

# 'float32' (exact, PE at 1/4 rate) or 'float32r' (tf32-like, full rate)
MM_DTYPE = "float32r"
TRACE = False
LAST_RESULT = None
LAST_NC = None

_nc_cache = {}


def _make_tile_context_cls():
    import concourse.mybir as mybir
    from concourse import tile
    from concourse.vector_clock import ScopedClock

    class TileContextFix(tile.TileContext):
        """This walrus build rejects >1 sync-wait per instruction.  Tile's
        scheduler freely assigns several.  Split the extras onto preceding
        NOPs on the same engine (same-engine program order makes this
        equivalent), and likewise chain the tail drain's waits."""

        _ws_counter = 0

        def _split_multi_waits(self):
            nc = self.nc
            for bb in nc.m.functions[0].blocks:
                insts = list(bb.instructions)
                if not any(
                    i.sync_info
                    and i.sync_info.on_wait
                    and len(i.sync_info.on_wait) > 1
                    for i in insts
                ):
                    continue
                new_seq = []
                for inst in insts:
                    si = inst.sync_info
                    waits = (
                        list(si.on_wait) if (si is not None and si.on_wait) else []
                    )
                    if len(waits) > 1:
                        for w in waits[:-1]:
                            TileContextFix._ws_counter += 1
                            nop = mybir.InstNoOp(
                                name=f"I-waitsplit-{TileContextFix._ws_counter}",
                                engine=inst.engine,
                            )
                            nop.sync_info = mybir.SyncInfo(
                                on_wait=[w], on_update=[]
                            )
                            new_seq.append(nop)
                        inst.sync_info = mybir.SyncInfo(
                            on_wait=[waits[-1]],
                            on_update=list(si.on_update) if si.on_update else [],
                        )
                    new_seq.append(inst)
                bb.instructions[:] = new_seq

        def _drain_and_barrier(self, tick_clock, wait_clock):
            self._split_multi_waits()
            drain_inst = self.nc.sync.drain()
            wait_clock.add_sem_waits(
                drain_inst.ins, ScopedClock({None: tick_clock.global_clock})
            )
            si = drain_inst.ins.sync_info
            waits = list(si.on_wait) if si is not None else []
            if len(waits) > 1:
                drain_inst.ins.sync_info = mybir.SyncInfo(
                    on_wait=waits[:1],
                    on_update=list(si.on_update) if si.on_update else [],
                )
                for w in waits[1:]:
                    extra = self.nc.sync.drain()
                    extra.ins.sync_info = mybir.SyncInfo(on_wait=[w], on_update=[])
            self.nc.all_engine_barrier()
            assert self.sems is not None
            popped = self.nc._tile_sem_poison_stack.pop()
            assert popped is self._sem_poison
            self.nc.clear_and_free_semaphores(list(self.sems.allocated().values()))
            self.nc.all_engine_barrier()

    return TileContextFix


def _tiles_of(M0, M1):
    """Row-tiles [(slot, col0, rows)]: full 128-row tiles + one ragged
    tile per slot."""
    tiles = []
    for s, (base, M) in enumerate(((0, M0), (M0, M1))):
        c = 0
        while c < M:
            r = min(P, M - c)
            tiles.append((s, base + c, r))
            c += r
    return tiles


def _build(M0, M1, mm_dtype):
    key = (M0, M1, mm_dtype)
    if key in _nc_cache:
        return _nc_cache[key]

    import concourse.bass as bass
    import concourse.mybir as mybir

    TileContextFix = _make_tile_context_cls()

    NTOT = M0 + M1
    # x columns padded so every 128-wide tile load stays in bounds
    NXPAD = M0 + P * (-(-M1 // P))
    nc = bass.Bass()
    mmdt = getattr(mybir.dt, mm_dtype)
    # x^T: [d, n] with routed rows as columns (slot0 block then slot1);
    # W pre-tiled on host as [slot][fb][dk] 128x512 contiguous blocks
    xT = nc.declare_dram_parameter("xT", [D, NXPAD], mmdt, isOutput=False)
    wt = nc.declare_dram_parameter(
        "wt", [SLOTS, NF, DK, P, FBLK], mmdt, isOutput=False
    )
    bias = nc.declare_dram_parameter("bias", [P, D], mybir.dt.float32, isOutput=False)
    out = nc.declare_dram_parameter("out", [NTOT, D], mybir.dt.float32, isOutput=True)

    tiles = _tiles_of(M0, M1)
    _SPLIT_LAST = globals().get("_SPLIT_LAST", 4)
    n_slot0 = sum(1 for s, _, _ in tiles if s == 0)
    WSPLIT = globals().get("_WSPLIT", 1)  # whole-W DMAs schedule best

    with TileContextFix(nc) as tc:
        with (
            tc.tile_pool(name="wpool", bufs=1) as wpool,
            tc.tile_pool(name="xpool", bufs=1) as xpool,
            tc.tile_pool(name="bpool", bufs=1) as bpool,
            tc.tile_pool(name="psum", bufs=6, space="PSUM") as pp,
            tc.tile_pool(name="opool", bufs=4) as op,
        ):
            # HWDGE descriptor generation is a serial ~625ns/DMA resource:
            # batch aggressively.  One DMA per x row-tile (all dk chunks,
            # always 128 cols -> full-width 512B descriptors), W in 1MB
            # half-blocks, one (ragged) store per row-tile.
            x_tiles = {}

            def load_x(t):
                _, c0, r = tiles[t]
                tl = xpool.tile([P, DK * P], mmdt, tag=f"x{t}")
                src = xT[:, c0 : c0 + P].rearrange("(dk p) m -> p dk m", p=P)
                nc.sync.dma_start(tl[:], src)
                x_tiles[t] = tl

            w_tiles = {}

            def load_w_half(s, fb, i):
                step = DK // WSPLIT
                tl = wpool.tile([P, step * FBLK], mmdt, tag=f"w{s}_{fb}_{i}")
                nc.sync.dma_start(
                    tl[:],
                    wt[s, fb, i * step : (i + 1) * step].rearrange(
                        "dk p f -> p dk f"
                    ),
                )
                w_tiles.setdefault((s, fb), [None] * WSPLIT)[i] = tl

            def load_w(s, fb):
                for i in range(WSPLIT):
                    load_w_half(s, fb, i)

            def w_slice(s, fb, dk):
                step = DK // WSPLIT
                tl = w_tiles[(s, fb)][dk // step]
                d = dk % step
                return tl[:, d * FBLK : (d + 1) * FBLK]

            bias_t = bpool.tile([P, D], mybir.dt.float32, tag="bias")

            # issue order = pipeline order: first psum group needs w(0,0)
            # and x(0); slot-1 weights land before the PE reaches the
            # slot-1 tiles, so the PE ramps early and stays fed.
            # bias first: the DVE evictions read bias_t, so it must be
            # resident before the first psum group retires or the psum
            # pool backs up and stalls the PE; later placements measure
            # strictly worse
            nc.sync.dma_start(bias_t[:], bias[:])
            load_w(0, 0)
            load_x(0)
            load_x(1)
            load_w(0, 1)
            for t in range(2, min(n_slot0 + 1, len(tiles))):
                load_x(t)
            load_w(1, 0)
            load_w(1, 1)
            for t in range(n_slot0 + 1, len(tiles)):
                load_x(t)

            for t, (s, c0, r) in enumerate(tiles):
                ot = op.tile([P, D], mybir.dt.float32, tag="o")
                for fb in range(NF):
                    ps = pp.tile([P, FBLK], mybir.dt.float32, tag="ps")
                    for dk in range(DK):
                        nc.tensor.matmul(
                            ps[:r, :],
                            x_tiles[t][:, dk * P : dk * P + r],
                            w_slice(s, fb, dk),
                            start=(dk == 0),
                            stop=(dk == DK - 1),
                        )
                    nc.vector.tensor_add(
                        ot[:r, fb * FBLK : (fb + 1) * FBLK],
                        ps[:r, :],
                        bias_t[:r, fb * FBLK : (fb + 1) * FBLK],
                    )
                # store issued from the otherwise-idle ACT engine so its
                # waits never head-of-line block the SP load stream; the
                # final two tiles store per-fb so their first halves
                # stream out before the last psum group retires
                if t >= len(tiles) - _SPLIT_LAST:
                    for fb in range(NF):
                        nc.scalar.dma_start(
                            out[c0 : c0 + r, fb * FBLK : (fb + 1) * FBLK],
                            ot[:r, fb * FBLK : (fb + 1) * FBLK],
                        )
                else:
                    nc.scalar.dma_start(out[c0 : c0 + r, :], ot[:r, :])

    _nc_cache[key] = nc
    return nc


def kernel(x, cond, W, b):
    from concourse.bass_utils import run_bass_kernel_spmd

    global LAST_RESULT, LAST_NC

    x = np.ascontiguousarray(np.asarray(x, dtype=np.float32))
    cond_i = np.asarray(cond).astype(np.int64)
    W = np.asarray(W, dtype=np.float32)
    b = np.asarray(b, dtype=np.float32)

    counts = np.bincount(cond_i, minlength=C)
    # Largest 8 experts -> slot 0, rest -> slot 1, so per-slot padding
    # (max count over that slot) is minimal.
    order = np.argsort(-counts, kind="stable")
    slot_experts = (order[:NCORES], order[NCORES:])
    M0 = max(1, int(counts[slot_experts[0]].max()))
    M1 = max(1, int(counts[slot_experts[1]].max()))
    NXPAD = M0 + P * (-(-M1 // P))

    nc = _build(M0, M1, MM_DTYPE)
    LAST_NC = nc

    bias_np = np.ascontiguousarray(
        np.broadcast_to(b.sum(axis=0).astype(np.float32), (P, D))
    )

    idx_by_e = [np.nonzero(cond_i == e)[0] for e in range(C)]
    in_maps = []
    placements = []
    for k in range(NCORES):
        xTk = np.zeros((D, NXPAD), np.float32)
        wTk = np.empty((SLOTS, D, D), np.float32)
        for s, col in enumerate((0, M0)):
            e = int(slot_experts[s][k])
            idx = idx_by_e[e]
            xTk[:, col : col + len(idx)] = x[idx].T
            wTk[s] = W[e].T
            placements.append((k, col, e))
        # [S, D, D] -> [S, NF, DK, 128, 512] contiguous blocks
        wtk = np.ascontiguousarray(
            wTk.reshape(SLOTS, DK, P, NF, FBLK).transpose(0, 3, 1, 2, 4)
        )
        in_maps.append({"xT": xTk, "wt": wtk, "bias": bias_np})

    res = run_bass_kernel_spmd(
        nc, in_maps, list(range(NCORES)), trace=TRACE
    )
    LAST_RESULT = res

    out_full = np.empty((B, D), np.float32)
    for k, col, e in placements:
        idx = idx_by_e[e]
        out_full[idx] = res.results[k]["out"][col : col + len(idx)]
    return out_full


if __name__ == "__main__":
    rng = np.random.default_rng(0)
    x = rng.standard_normal((B, D), dtype=np.float32)
    cond = rng.integers(0, C, size=B).astype(np.int64)
    W = (rng.standard_normal((C, D, D), dtype=np.float32) / np.sqrt(D)).astype(
        np.float32
    )
    b = (rng.standard_normal((C, D), dtype=np.float32) * 0.02).astype(np.float32)
    got = kernel(x, cond, W, b)
    want = np.empty((B, D), np.float32)
    for e in range(C):
        idx = np.nonzero(cond == e)[0]
        want[idx] = x[idx] @ W[e].T
    want += b.sum(0)
    denom = np.abs(want).max()
    print("max abs err:", np.abs(got - want).max(), "denom:", denom)
    print("rel err:", np.abs(got - want).max() / denom)
